# revision 15
# baseline (speedup 1.0000x reference)
"""Trainium2 Bass kernel for nn_AdaptiveTransformerBlock (B=2, T=2048, D=1024).

Strategy (8 NeuronCores, one chip):
  - x (both batches) is replicated to every core in bf16, transposed
    feature-major [D, B*T]. Every core computes rmsnorm1 + its 2 heads'
    q/k/v (head tensor-parallelism, 16 heads / 8 cores), and causal
    attention for those 2 heads over both batches.
  - One 8-rank AllToAll (1 MB) reshards attention output o from
    head-parallel to token-parallel (each core owns a 512-token block).
  - Out-proj, residual, rmsnorm2, SwiGLU FFN and the halting head then
    run fully locally on the core's 512 tokens with full (streamed)
    weights. Outputs are token-sharded; the host reassembles.

All matmuls run in bf16 with fp32 PSUM accumulation; the residual path
stays fp32. g1/g2 are folded into Wqkv/W1/W2 rows host-side; the 1/8
attention scale is folded into Wq.
"""

import sys

sys.path.insert(0, "/opt/trn_rl_repo")

import numpy as np
import ml_dtypes

import concourse.bass as bass
import concourse.mybir as mybir
import concourse.tile as tile
from concourse import bacc
from concourse.bass_utils import run_bass_kernel_spmd

BF16 = mybir.dt.bfloat16
F32 = mybir.dt.float32
AF = mybir.ActivationFunctionType

P = 128
D = 1024
B = 2
T = 2048
TT = B * T  # 4096
TB = 512  # tokens owned per core
KD = D // P  # 8
NCH = TT // 512  # 8 global 512-token chunks
NQC = T // 512  # 4 q-chunks per batch
DFF = 2730
DFFP = 2816  # padded to 22*128
MFF = DFFP // P  # 22
DH = 256
EPS = 1e-6
HALT_THRESHOLD = 0.99

N_CORES = 8

LAST_EXEC_NS = None
_CACHE = {}


def _build():
    nc = bacc.Bacc(
        "TRN2",
        target_bir_lowering=False,
        debug=False,
        enable_asserts=False,
        num_devices=N_CORES,
    )

    # ---- dram parameters ----
    xt_ext = nc.dram_tensor("xt", [NCH, KD, P, 512], BF16, kind="ExternalInput")
    xo_ext = nc.dram_tensor("xo", [D, TB], F32, kind="ExternalInput")
    wqkv_ext = nc.dram_tensor("wqkv", [D, 384], BF16, kind="ExternalInput")
    wo_ext2 = nc.dram_tensor("wo", [KD, P, KD, P], BF16, kind="ExternalInput")
    w1_ext = nc.dram_tensor("w1", [MFF, P, KD, P], BF16, kind="ExternalInput")
    w2_ext = nc.dram_tensor("w2", [MFF, P, KD, P], BF16, kind="ExternalInput")
    w3_ext = nc.dram_tensor("w3", [KD, P, MFF, P], BF16, kind="ExternalInput")
    wh1_ext = nc.dram_tensor("wh1", [D, DH], BF16, kind="ExternalInput")
    bh1_ext = nc.dram_tensor("bh1", [DH, 1], F32, kind="ExternalInput")
    wh2_ext = nc.dram_tensor("wh2", [DH, 1], BF16, kind="ExternalInput")
    bh2_ext = nc.dram_tensor("bh2", [1, 1], F32, kind="ExternalInput")
    ch_ext = nc.dram_tensor("ch", [1, TB], F32, kind="ExternalInput")
    triu_ext = nc.dram_tensor("triu", [P, P], BF16, kind="ExternalInput")

    xt_out = nc.dram_tensor("xt_out", [D, TB], F32, kind="ExternalOutput")
    halt_out = nc.dram_tensor("halt_o", [1, TB], F32, kind="ExternalOutput")
    ncum_out = nc.dram_tensor("ncum_o", [1, TB], F32, kind="ExternalOutput")

    a2aA_in = nc.dram_tensor("a2aA_in", [N_CORES, 64, TB], BF16)
    a2aA_out = nc.dram_tensor("a2aA_out", [N_CORES, 64, TB], BF16)
    a2aB_in = nc.dram_tensor("a2aB_in", [N_CORES, 64, TB], BF16)
    a2aB_out = nc.dram_tensor("a2aB_out", [N_CORES, 64, TB], BF16)

    with tile.TileContext(nc) as tc:
        with tc.tile_pool(name="pers", bufs=1) as pers:
            # persistent tiles (live across phases)
            xo = [pers.tile([P, TB], F32, tag=f"xo{k}", name=f"xo{k}") for k in range(KD)]
            wqkv_sb = [pers.tile([P, 384], BF16, tag=f"wqkv{k}", name=f"wqkv{k}") for k in range(KD)]
            wh1_sb = [pers.tile([P, DH], BF16, tag=f"wh1{k}", name=f"wh1{k}") for k in range(KD)]
            wh2_sb = [pers.tile([P, 1], BF16, tag=f"wh2{k}", name=f"wh2{k}") for k in range(2)]
            bh1_sb = [pers.tile([P, 1], F32, tag=f"bh1{k}", name=f"bh1{k}") for k in range(2)]
            bh2_sb = pers.tile([1, 1], F32, tag="bh2")
            ch_sb = pers.tile([1, TB], F32, tag="ch")
            triu_sb = pers.tile([P, P], BF16, tag="triu")
            ones_mat = pers.tile([P, P], BF16, tag="ones_mat")
            eps_sb = pers.tile([P, 1], F32, tag="eps")
            oTfA = [pers.tile([P, TB], BF16, tag=f"oTfA{k}", name=f"oTfA{k}") for k in range(KD)]
            oTfB = [pers.tile([P, TB], BF16, tag=f"oTfB{k}", name=f"oTfB{k}") for k in range(KD)]
            h2 = [pers.tile([P, TB], BF16, tag=f"h2{k}", name=f"h2{k}") for k in range(KD)]
            halt_sb = pers.tile([1, TB], F32, tag="halt")
            still_sb = pers.tile([1, TB], F32, tag="still")
            ncum_sb = pers.tile([1, TB], F32, tag="ncum")

            # ---- attention-scope tensors (freed before the FFN phase) ----
            ab_cm = tc.tile_pool(name="ab", bufs=1)
            ab = ab_cm.__enter__()
            qT = [ab.tile([P, 512], BF16, tag=f"qT{g}", name=f"qT{g}") for g in range(NCH)]
            kTz = [
                [ab.tile([P, 512], BF16, tag=f"kTz{hl}_{g}", name=f"kTz{hl}_{g}") for g in range(NCH)]
                for hl in range(2)
            ]
            vsb = [ab.tile([P, 256], BF16, tag=f"v{t}", name=f"v{t}") for t in range(TT // P)]
            oT = [ab.tile([P, 512], BF16, tag=f"oT{g}", name=f"oT{g}") for g in range(NCH)]

            # ---- phase A: rmsnorm1 (in-place into xh) + qkv ----
            xp_cm = tc.tile_pool(name="xp", bufs=1)
            xp = xp_cm.__enter__()
            xh = [
                [xp.tile([P, 512], BF16, tag=f"xh{k}_{n}", name=f"xh{k}_{n}") for n in range(NCH)]
                for k in range(KD)
            ]
            # x tiles first: they gate the whole pipeline
            for n in range(NCH):
                for k in range(KD):
                    nc.sync.dma_start(out=xh[k][n][:], in_=xt_ext[n, k])
            nc.vector.memset(eps_sb[:], EPS)
            nc.vector.memset(ones_mat[:], 1.0)
            for k in range(KD):
                nc.sync.dma_start(out=wqkv_sb[k][:], in_=wqkv_ext[k * P : (k + 1) * P, :])
                nc.sync.dma_start(out=xo[k][:], in_=xo_ext[k * P : (k + 1) * P, :])
            for k in range(KD):
                nc.sync.dma_start(out=wh1_sb[k][:], in_=wh1_ext[k * P : (k + 1) * P, :])
            for k in range(2):
                nc.sync.dma_start(out=wh2_sb[k][:], in_=wh2_ext[k * P : (k + 1) * P, :])
                nc.sync.dma_start(out=bh1_sb[k][:], in_=bh1_ext[k * P : (k + 1) * P, :])
            nc.sync.dma_start(out=bh2_sb[:], in_=bh2_ext[:])
            nc.sync.dma_start(out=ch_sb[:], in_=ch_ext[:])
            nc.sync.dma_start(out=triu_sb[:], in_=triu_ext[:])
            with (
                tc.tile_pool(name="sqp", bufs=3) as sqp,
                tc.tile_pool(name="smallA", bufs=2) as smallA,
                tc.tile_pool(name="rbsp", bufs=3) as rbsp,
                tc.tile_pool(name="ssp", bufs=2, space="PSUM") as ssp,
                tc.tile_pool(name="qkvp", bufs=2, space="PSUM") as qkvp,
            ):
                for n in range(NCH):
                    ss_t = ssp.tile([P, 512], F32)
                    for k in range(KD):
                        sq = sqp.tile([P, 512], BF16)
                        if k % 2 == 0:
                            nc.scalar.square(sq[:], xh[k][n][:])
                        else:
                            nc.vector.tensor_mul(sq[:], xh[k][n][:], xh[k][n][:])
                        nc.tensor.matmul(
                            ss_t[:],
                            lhsT=ones_mat[:],
                            rhs=sq[:],
                            start=(k == 0),
                            stop=(k == KD - 1),
                        )
                    std_t = smallA.tile([P, 512], F32, tag="stdA")
                    nc.scalar.activation(
                        out=std_t[:], in_=ss_t[:], func=AF.Sqrt, bias=eps_sb[:], scale=1.0 / D
                    )
                    rstd_t = smallA.tile([P, 512], F32, tag="rstdA")
                    nc.vector.reciprocal_approx_fast(rstd_t[:], std_t[:])
                    rb_sb = rbsp.tile([P, 512], BF16)
                    nc.vector.tensor_copy(rb_sb[:], rstd_t[:])
                    for k in range(KD):
                        nc.vector.tensor_mul(xh[k][n][:], xh[k][n][:], rb_sb[:])

                # q, k (feature-major); k is written zero-padded per head so the
                # scores matmul can run with K=128 (block-diagonal stationary)
                for n in range(NCH):
                    for which in range(2):
                        ps = qkvp.tile([P, 512], F32, tag="qk_ps")
                        for k in range(KD):
                            nc.tensor.matmul(
                                ps[:],
                                lhsT=wqkv_sb[k][:, which * P : (which + 1) * P],
                                rhs=xh[k][n][:],
                                start=(k == 0),
                                stop=(k == KD - 1),
                            )
                        if which == 0:
                            nc.scalar.copy(qT[n][:], ps[:])
                        else:
                            nc.gpsimd.memset(kTz[0][n][64:128, :], 0.0)
                            nc.scalar.copy(kTz[0][n][0:64, :], ps[0:64, :])
                            nc.gpsimd.memset(kTz[1][n][0:64, :], 0.0)
                            nc.scalar.copy(kTz[1][n][64:128, :], ps[64:128, :])
                # v (token-major), zero-padded to 128 cols/head with a ones col
                for t in range(TT // P):
                    n, off = t // 4, (t % 4) * P
                    ps = qkvp.tile([P, P], F32, tag="v_ps")
                    for k in range(KD):
                        nc.tensor.matmul(
                            ps[:],
                            lhsT=xh[k][n][:, off : off + P],
                            rhs=wqkv_sb[k][:, 256:384],
                            start=(k == 0),
                            stop=(k == KD - 1),
                        )
                    v3 = vsb[t][:].rearrange("p (h x) -> p h x", h=2)
                    nc.scalar.copy(v3[:, :, 0:64], ps[:].rearrange("p (h x) -> p h x", h=2))
                    nc.gpsimd.memset(v3[:, :, 64:128], 1.0)
            xp_cm.__exit__(None, None, None)

            # ---- phase B: attention; A2A split per local head so the first
            # collective overlaps the second head's attention compute ----
            with (
                tc.tile_pool(name="expp", bufs=6) as expp,
                tc.tile_pool(name="smallB", bufs=2) as smallB,
                tc.tile_pool(name="sp", bufs=4, space="PSUM") as sp,
                tc.tile_pool(name="op", bufs=3, space="PSUM") as op,
            ):
                for hl in range(2):
                    hp = 64 * hl
                    for b in range(B):
                        for qc in range(NQC):
                            g_q = NQC * b + qc
                            o_ps = op.tile([P, 512], F32)
                            nkt = 4 * qc + 4
                            for kt in range(nkt):
                                tok0 = T * b + P * kt
                                g_k, off_k = tok0 // 512, tok0 % 512
                                s_ps = sp.tile([P, 512], F32)
                                nc.tensor.matmul(
                                    s_ps[:],
                                    lhsT=kTz[hl][g_k][:, off_k : off_k + P],
                                    rhs=qT[g_q][:],
                                    start=True,
                                    stop=True,
                                )
                                e_t = expp.tile([P, 512], BF16)
                                j = kt - 4 * qc
                                if j < 0:
                                    nc.scalar.activation(
                                        out=e_t[:], in_=s_ps[:], func=AF.Exp
                                    )
                                else:
                                    if j > 0:
                                        nc.vector.memset(e_t[:, 0 : P * j], 0.0)
                                    nc.scalar.activation(
                                        out=e_t[:, P * j : 512],
                                        in_=s_ps[:, P * j : 512],
                                        func=AF.Exp,
                                    )
                                    nc.vector.tensor_mul(
                                        e_t[:, P * j : P * (j + 1)],
                                        e_t[:, P * j : P * (j + 1)],
                                        triu_sb[:],
                                    )
                                nc.tensor.matmul(
                                    o_ps[:],
                                    lhsT=vsb[16 * b + kt][:, P * hl : P * (hl + 1)],
                                    rhs=e_t[:],
                                    start=(kt == 0),
                                    stop=(kt == nkt - 1),
                                )
                            den = smallB.tile([64, 512], F32, tag="den")
                            nc.vector.tensor_copy(den[:], o_ps[64:128, :])
                            r_t = smallB.tile([64, 512], F32, tag="r")
                            nc.vector.reciprocal_approx_fast(r_t[:], den[:])
                            ob = smallB.tile([64, 512], BF16, tag="ob")
                            nc.vector.tensor_copy(ob[:], o_ps[0:64, :])
                            nc.vector.tensor_mul(
                                oT[g_q][hp : hp + 64, :], ob[:], r_t[:]
                            )
                            if hl == 0:
                                nc.gpsimd.dma_start(
                                    out=a2aA_in[g_q], in_=oT[g_q][0:64, :]
                                )
                            else:
                                nc.gpsimd.dma_start(
                                    out=a2aB_in[g_q], in_=oT[g_q][64:128, :]
                                )
                    # resharding collective for this head-half
                    if hl == 0:
                        nc.gpsimd.collective_compute(
                            "AllToAll",
                            mybir.AluOpType.bypass,
                            replica_groups=[list(range(N_CORES))],
                            ins=[a2aA_in[:]],
                            outs=[a2aA_out[:]],
                        )
                        for k in range(KD):
                            nc.gpsimd.memset(oTfA[k][64:128, :], 0.0)
                            nc.gpsimd.dma_start(out=oTfA[k][0:64, :], in_=a2aA_out[k])
                    else:
                        nc.gpsimd.collective_compute(
                            "AllToAll",
                            mybir.AluOpType.bypass,
                            replica_groups=[list(range(N_CORES))],
                            ins=[a2aB_in[:]],
                            outs=[a2aB_out[:]],
                        )
                        for k in range(KD):
                            nc.gpsimd.memset(oTfB[k][0:64, :], 0.0)
                            nc.gpsimd.dma_start(out=oTfB[k][64:128, :], in_=a2aB_out[k])
            ab_cm.__exit__(None, None, None)

            # ---- phase C: out-proj + residual + rmsnorm2 ----
            with (
                tc.tile_pool(name="sq2p", bufs=3) as sq2p,
                tc.tile_pool(name="wosp", bufs=3) as wosp,
                tc.tile_pool(name="smallC", bufs=2) as smallC,
                tc.tile_pool(name="wop", bufs=2, space="PSUM") as wop,
                tc.tile_pool(name="ss2p", bufs=1, space="PSUM") as ss2p,
            ):
                for m in range(KD):
                    wot = wosp.tile([P, KD, P], BF16, tag="wos")
                    nc.sync.dma_start(out=wot[:], in_=wo_ext2[m])
                    w_ps = wop.tile([P, 512], F32)
                    for k in range(KD):
                        nc.tensor.matmul(
                            w_ps[:],
                            lhsT=wot[:, k, :],
                            rhs=oTfA[k][:],
                            start=(k == 0),
                            stop=(k == KD - 1),
                        )
                    nc.vector.tensor_add(xo[m][:], w_ps[:], xo[m][:])
                ss_t = ss2p.tile([P, 512], F32)
                for m in range(KD):
                    wot = wosp.tile([P, KD, P], BF16, tag="wosB")
                    nc.sync.dma_start(out=wot[:], in_=wo_ext2[m])
                    w_ps = wop.tile([P, 512], F32)
                    for k in range(KD):
                        nc.tensor.matmul(
                            w_ps[:],
                            lhsT=wot[:, k, :],
                            rhs=oTfB[k][:],
                            start=(k == 0),
                            stop=(k == KD - 1),
                        )
                    nc.vector.tensor_add(xo[m][:], w_ps[:], xo[m][:])
                    sq = sq2p.tile([P, 512], BF16)
                    if m % 2 == 0:
                        nc.scalar.square(sq[:], xo[m][:])
                    else:
                        nc.vector.tensor_mul(sq[:], xo[m][:], xo[m][:])
                    nc.tensor.matmul(
                        ss_t[:],
                        lhsT=ones_mat[:],
                        rhs=sq[:],
                        start=(m == 0),
                        stop=(m == KD - 1),
                    )
                std_t = smallC.tile([P, 512], F32, tag="stdC")
                nc.scalar.activation(
                    out=std_t[:], in_=ss_t[:], func=AF.Sqrt, bias=eps_sb[:], scale=1.0 / D
                )
                rstd_t = smallC.tile([P, 512], F32, tag="rstdC")
                nc.vector.reciprocal_approx_fast(rstd_t[:], std_t[:])
                rb_sb = smallC.tile([P, 512], BF16, tag="rb2sb")
                nc.vector.tensor_copy(rb_sb[:], rstd_t[:])
                for k in range(KD):
                    nc.vector.tensor_mul(h2[k][:], xo[k][:], rb_sb[:])

            # ---- phase D: FFN (full weights, streamed) ----
            with (
                tc.tile_pool(name="gffp", bufs=1) as gffp,
                tc.tile_pool(name="wffp", bufs=3) as wffp,
                tc.tile_pool(name="w3p", bufs=2) as w3p,
                tc.tile_pool(name="silp", bufs=3) as silp,
                tc.tile_pool(name="up", bufs=2, space="PSUM") as up,
                tc.tile_pool(name="w2psum", bufs=2, space="PSUM") as w2psum,
                tc.tile_pool(name="fp", bufs=2, space="PSUM") as fp,
            ):
                gff = [gffp.tile([P, TB], BF16, tag=f"gff{m}", name=f"gff{m}") for m in range(MFF)]
                for m in range(MFF):
                    w1t = wffp.tile([P, KD, P], BF16, tag="w1s")
                    nc.sync.dma_start(out=w1t[:], in_=w1_ext[m])
                    u_ps = up.tile([P, 512], F32)
                    for k in range(KD):
                        nc.tensor.matmul(
                            u_ps[:],
                            lhsT=w1t[:, k, :],
                            rhs=h2[k][:],
                            start=(k == 0),
                            stop=(k == KD - 1),
                        )
                    w2t = wffp.tile([P, KD, P], BF16, tag="w2s")
                    nc.sync.dma_start(out=w2t[:], in_=w2_ext[m])
                    w2_ps = w2psum.tile([P, 512], F32)
                    for k in range(KD):
                        nc.tensor.matmul(
                            w2_ps[:],
                            lhsT=w2t[:, k, :],
                            rhs=h2[k][:],
                            start=(k == 0),
                            stop=(k == KD - 1),
                        )
                    sil = silp.tile([P, 512], BF16)
                    nc.scalar.activation(out=sil[:], in_=u_ps[:], func=AF.Silu)
                    nc.vector.tensor_mul(gff[m][:], w2_ps[:], sil[:])
                for m3 in range(KD):
                    w3t = w3p.tile([P, MFF, P], BF16, tag="w3s")
                    nc.sync.dma_start(out=w3t[:], in_=w3_ext[m3])
                    f_ps = fp.tile([P, 512], F32)
                    for k3 in range(MFF):
                        nc.tensor.matmul(
                            f_ps[:],
                            lhsT=w3t[:, k3, :],
                            rhs=gff[k3][:],
                            start=(k3 == 0),
                            stop=(k3 == MFF - 1),
                        )
                    nc.vector.tensor_add(xo[m3][:], f_ps[:], xo[m3][:])
                    nc.vector.tensor_copy(h2[m3][:], xo[m3][:])  # x3 in bf16
                    nc.sync.dma_start(
                        out=xt_out[m3 * P : (m3 + 1) * P, :], in_=xo[m3][:]
                    )

            # ---- phase E: halting head ----
            with (
                tc.tile_pool(name="hps", bufs=2, space="PSUM") as hps,
                tc.tile_pool(name="hpp", bufs=1, space="PSUM") as hpp,
                tc.tile_pool(name="smallE", bufs=2) as smallE,
            ):
                hh = [smallE.tile([P, TB], BF16, tag=f"hh{k}", name=f"hh{k}") for k in range(2)]
                for mh in range(2):
                    h_ps = hps.tile([P, 512], F32)
                    for k in range(KD):
                        nc.tensor.matmul(
                            h_ps[:],
                            lhsT=wh1_sb[k][:, mh * P : (mh + 1) * P],
                            rhs=h2[k][:],
                            start=(k == 0),
                            stop=(k == KD - 1),
                        )
                    nc.scalar.activation(
                        out=hh[mh][:], in_=h_ps[:], func=AF.Gelu, bias=bh1_sb[mh][:]
                    )
                hp_ps = hpp.tile([1, 512], F32)
                for k2 in range(2):
                    nc.tensor.matmul(
                        hp_ps[:],
                        lhsT=wh2_sb[k2][:],
                        rhs=hh[k2][:],
                        start=(k2 == 0),
                        stop=(k2 == 1),
                    )
                nc.scalar.activation(
                    out=halt_sb[:], in_=hp_ps[:], func=AF.Sigmoid, bias=bh2_sb[:]
                )
                nc.vector.tensor_scalar(
                    out=still_sb[:],
                    in0=ch_sb[:],
                    scalar1=HALT_THRESHOLD,
                    scalar2=None,
                    op0=mybir.AluOpType.is_lt,
                )
                tmp = smallE.tile([1, TB], F32)
                nc.vector.tensor_mul(tmp[:], halt_sb[:], still_sb[:])
                nc.vector.tensor_add(ncum_sb[:], tmp[:], ch_sb[:])
                nc.sync.dma_start(out=halt_out[:], in_=halt_sb[:])
                nc.sync.dma_start(out=ncum_out[:], in_=ncum_sb[:])

    nc.compile()
    return nc


def _prep_in_maps(inputs):
    bf = ml_dtypes.bfloat16
    x = np.asarray(inputs["x"], np.float32)
    ch = np.asarray(inputs["cumulative_halt"], np.float32)
    g1 = np.asarray(inputs["g1"], np.float32)
    g2 = np.asarray(inputs["g2"], np.float32)
    Wqkv = np.asarray(inputs["Wqkv"], np.float32) * g1[:, None]
    Wo = np.asarray(inputs["Wo"], np.float32)
    W1 = np.asarray(inputs["W1"], np.float32) * g2[:, None]
    W2 = np.asarray(inputs["W2"], np.float32) * g2[:, None]
    W3 = np.asarray(inputs["W3"], np.float32)
    Wh1 = np.asarray(inputs["Wh1"], np.float32)
    bh1 = np.asarray(inputs["bh1"], np.float32)
    Wh2 = np.asarray(inputs["Wh2"], np.float32)
    bh2 = np.asarray(inputs["bh2"], np.float32)

    xt = np.ascontiguousarray(
        x.reshape(TT, D).T.reshape(KD, P, NCH, 512).transpose(2, 0, 1, 3)
    ).astype(bf)  # [NCH, KD, P, 512] chunk-major tiles

    W1p = np.zeros((D, DFFP), np.float32)
    W1p[:, :DFF] = W1
    W2p = np.zeros((D, DFFP), np.float32)
    W2p[:, :DFF] = W2
    W3p = np.zeros((DFFP, D), np.float32)
    W3p[:DFF, :] = W3
    w1t = np.ascontiguousarray(
        W1p.reshape(KD, P, MFF, P).transpose(2, 1, 0, 3)
    ).astype(bf)
    w2t = np.ascontiguousarray(
        W2p.reshape(KD, P, MFF, P).transpose(2, 1, 0, 3)
    ).astype(bf)
    w3t = np.ascontiguousarray(
        W3p.reshape(MFF, P, KD, P).transpose(2, 1, 0, 3)
    ).astype(bf)

    wo_bf = np.ascontiguousarray(
        Wo.reshape(KD, P, KD, P).transpose(2, 1, 0, 3)
    ).astype(bf)
    wh1_bf = Wh1.astype(bf)
    wh2_bf = Wh2.reshape(DH, 1).astype(bf)
    bh1_c = bh1.reshape(DH, 1).astype(np.float32)
    bh2_c = bh2.reshape(1, 1).astype(np.float32)
    triu = np.triu(np.ones((P, P), np.float32)).astype(bf)

    in_maps = []
    for c in range(N_CORES):
        b, blk = c // 4, c % 4
        # q/k/v columns for heads 2c, 2c+1; fold 1/8 score scale into Wq
        qcols = Wqkv[:, P * c : P * (c + 1)] * 0.125
        kcols = Wqkv[:, D + P * c : D + P * (c + 1)]
        vcols = Wqkv[:, 2 * D + P * c : 2 * D + P * (c + 1)]
        wqkv_c = np.concatenate([qcols, kcols, vcols], axis=1).astype(bf)
        xo_c = np.ascontiguousarray(
            x[b, TB * blk : TB * (blk + 1), :].T
        ).astype(np.float32)
        ch_c = np.ascontiguousarray(
            ch[b, TB * blk : TB * (blk + 1), 0].reshape(1, TB)
        ).astype(np.float32)
        in_maps.append(
            {
                "xt": xt,
                "xo": xo_c,
                "wqkv": wqkv_c,
                "wo": wo_bf,
                "w1": w1t,
                "w2": w2t,
                "w3": w3t,
                "wh1": wh1_bf,
                "bh1": bh1_c,
                "wh2": wh2_bf,
                "bh2": bh2_c,
                "ch": ch_c,
                "triu": triu,
            }
        )
    return in_maps


def kernel(**inputs):
    global LAST_EXEC_NS
    if "nc" not in _CACHE:
        _CACHE["nc"] = _build()
    nc = _CACHE["nc"]
    in_maps = _prep_in_maps(inputs)
    import os

    trace = bool(os.environ.get("KERNEL_TRACE"))
    res = run_bass_kernel_spmd(
        nc, in_maps, core_ids=list(range(N_CORES)), trace=trace
    )
    LAST_EXEC_NS = res.exec_time_ns
    _CACHE["last_results"] = res

    x_out = np.empty((B, T, D), np.float32)
    halt = np.empty((B, T, 1), np.float32)
    ncum = np.empty((B, T, 1), np.float32)
    for c in range(N_CORES):
        b, blk = c // 4, c % 4
        r = res.results[c]
        x_out[b, TB * blk : TB * (blk + 1), :] = r["xt_out"].T
        halt[b, TB * blk : TB * (blk + 1), 0] = r["halt_o"][0]
        ncum[b, TB * blk : TB * (blk + 1), 0] = r["ncum_o"][0]
    return (x_out, halt, ncum)


# revision 16
# speedup vs baseline: 1.0569x; 1.0569x over previous
"""Trainium2 Bass kernel for nn_AdaptiveTransformerBlock (B=2, T=2048, D=1024).

Strategy (8 NeuronCores, one chip):
  - x (both batches) is replicated to every core in bf16, transposed
    feature-major [D, B*T]. Every core computes rmsnorm1 + its 2 heads'
    q/k/v (head tensor-parallelism, 16 heads / 8 cores), and causal
    attention for those 2 heads over both batches.
  - One 8-rank AllToAll (1 MB) reshards attention output o from
    head-parallel to token-parallel (each core owns a 512-token block).
  - Out-proj, residual, rmsnorm2, SwiGLU FFN and the halting head then
    run fully locally on the core's 512 tokens with full (streamed)
    weights. Outputs are token-sharded; the host reassembles.

All matmuls run in bf16 with fp32 PSUM accumulation; the residual path
stays fp32. g1/g2 are folded into Wqkv/W1/W2 rows host-side; the 1/8
attention scale is folded into Wq.
"""

import sys

sys.path.insert(0, "/opt/trn_rl_repo")

import numpy as np
import ml_dtypes

import concourse.bass as bass
import concourse.mybir as mybir
import concourse.tile as tile
from concourse import bacc
from concourse.bass_utils import run_bass_kernel_spmd

BF16 = mybir.dt.bfloat16
F32 = mybir.dt.float32
AF = mybir.ActivationFunctionType

P = 128
D = 1024
B = 2
T = 2048
TT = B * T  # 4096
TB = 512  # tokens owned per core
KD = D // P  # 8
NCH = TT // 512  # 8 global 512-token chunks
NQC = T // 512  # 4 q-chunks per batch
DFF = 2730
DFFP = 2816  # padded to 22*128
MFF = DFFP // P  # 22
DH = 256
EPS = 1e-6
HALT_THRESHOLD = 0.99

N_CORES = 8

LAST_EXEC_NS = None
_CACHE = {}


def _build():
    nc = bacc.Bacc(
        "TRN2",
        target_bir_lowering=False,
        debug=False,
        enable_asserts=False,
        num_devices=N_CORES,
    )

    # ---- dram parameters ----
    xt_ext = nc.dram_tensor("xt", [NCH, KD, P, 512], BF16, kind="ExternalInput")
    xo_ext = nc.dram_tensor("xo", [D, TB], F32, kind="ExternalInput")
    wqkv_ext = nc.dram_tensor("wqkv", [D, 384], BF16, kind="ExternalInput")
    wo_ext2 = nc.dram_tensor("wo", [KD, P, KD, P], BF16, kind="ExternalInput")
    w1_ext = nc.dram_tensor("w1", [MFF, P, KD, P], BF16, kind="ExternalInput")
    w2_ext = nc.dram_tensor("w2", [MFF, P, KD, P], BF16, kind="ExternalInput")
    w3_ext = nc.dram_tensor("w3", [KD, P, MFF, P], BF16, kind="ExternalInput")
    wh1_ext = nc.dram_tensor("wh1", [D, DH], BF16, kind="ExternalInput")
    bh1_ext = nc.dram_tensor("bh1", [DH, 1], F32, kind="ExternalInput")
    wh2_ext = nc.dram_tensor("wh2", [DH, 1], BF16, kind="ExternalInput")
    bh2_ext = nc.dram_tensor("bh2", [1, 1], F32, kind="ExternalInput")
    ch_ext = nc.dram_tensor("ch", [1, TB], F32, kind="ExternalInput")
    triu_ext = nc.dram_tensor("triu", [P, P], BF16, kind="ExternalInput")

    xt_out = nc.dram_tensor("xt_out", [D, TB], F32, kind="ExternalOutput")
    halt_out = nc.dram_tensor("halt_o", [1, TB], F32, kind="ExternalOutput")
    ncum_out = nc.dram_tensor("ncum_o", [1, TB], F32, kind="ExternalOutput")

    warm_in = nc.dram_tensor("warm_in", [N_CORES, 1, 128], BF16)
    warm_out = nc.dram_tensor("warm_out", [N_CORES, 1, 128], BF16)
    a2aA_in = nc.dram_tensor("a2aA_in", [N_CORES, 64, TB], BF16)
    a2aA_out = nc.dram_tensor("a2aA_out", [N_CORES, 64, TB], BF16)
    a2aB_in = nc.dram_tensor("a2aB_in", [N_CORES, 64, TB], BF16)
    a2aB_out = nc.dram_tensor("a2aB_out", [N_CORES, 64, TB], BF16)

    with tile.TileContext(nc) as tc:
        with tc.tile_pool(name="pers", bufs=1) as pers:
            # persistent tiles (live across phases)
            xo = [pers.tile([P, TB], F32, tag=f"xo{k}", name=f"xo{k}") for k in range(KD)]
            wqkv_sb = [pers.tile([P, 384], BF16, tag=f"wqkv{k}", name=f"wqkv{k}") for k in range(KD)]
            wh1_sb = [pers.tile([P, DH], BF16, tag=f"wh1{k}", name=f"wh1{k}") for k in range(KD)]
            wh2_sb = [pers.tile([P, 1], BF16, tag=f"wh2{k}", name=f"wh2{k}") for k in range(2)]
            bh1_sb = [pers.tile([P, 1], F32, tag=f"bh1{k}", name=f"bh1{k}") for k in range(2)]
            bh2_sb = pers.tile([1, 1], F32, tag="bh2")
            ch_sb = pers.tile([1, TB], F32, tag="ch")
            triu_sb = pers.tile([P, P], BF16, tag="triu")
            ones_mat = pers.tile([P, P], BF16, tag="ones_mat")
            eps_sb = pers.tile([P, 1], F32, tag="eps")
            oTfA = [pers.tile([P, TB], BF16, tag=f"oTfA{k}", name=f"oTfA{k}") for k in range(KD)]
            oTfB = [pers.tile([P, TB], BF16, tag=f"oTfB{k}", name=f"oTfB{k}") for k in range(KD)]
            h2 = [pers.tile([P, TB], BF16, tag=f"h2{k}", name=f"h2{k}") for k in range(KD)]
            halt_sb = pers.tile([1, TB], F32, tag="halt")
            still_sb = pers.tile([1, TB], F32, tag="still")
            ncum_sb = pers.tile([1, TB], F32, tag="ncum")

            # ---- attention-scope tensors (freed before the FFN phase) ----
            ab_cm = tc.tile_pool(name="ab", bufs=1)
            ab = ab_cm.__enter__()
            qT = [ab.tile([P, 512], BF16, tag=f"qT{g}", name=f"qT{g}") for g in range(NCH)]
            kTz = [
                [ab.tile([P, 512], BF16, tag=f"kTz{hl}_{g}", name=f"kTz{hl}_{g}") for g in range(NCH)]
                for hl in range(2)
            ]
            vsb = [ab.tile([P, 256], BF16, tag=f"v{t}", name=f"v{t}") for t in range(TT // P)]
            oT = [ab.tile([P, 512], BF16, tag=f"oT{g}", name=f"oT{g}") for g in range(NCH)]

            # ---- phase A: rmsnorm1 (in-place into xh) + qkv ----
            xp_cm = tc.tile_pool(name="xp", bufs=1)
            xp = xp_cm.__enter__()
            xh = [
                [xp.tile([P, 512], BF16, tag=f"xh{k}_{n}", name=f"xh{k}_{n}") for n in range(NCH)]
                for k in range(KD)
            ]
            # x tiles first: they gate the whole pipeline
            for n in range(NCH):
                for k in range(KD):
                    nc.sync.dma_start(out=xh[k][n][:], in_=xt_ext[n, k])
            nc.vector.memset(eps_sb[:], EPS)
            nc.vector.memset(ones_mat[:], 1.0)
            # tiny warm-up collective: absorbs first-collective setup cost and
            # aligns the cores while input DMAs stream
            warm_sb = pers.tile([1, 128], BF16, tag="warm")
            nc.vector.memset(warm_sb[:], 0.0)
            for g in range(N_CORES):
                nc.gpsimd.dma_start(out=warm_in[g], in_=warm_sb[:])
            nc.gpsimd.collective_compute(
                "AllToAll",
                mybir.AluOpType.bypass,
                replica_groups=[list(range(N_CORES))],
                ins=[warm_in[:]],
                outs=[warm_out[:]],
            )
            for k in range(KD):
                nc.sync.dma_start(out=wqkv_sb[k][:], in_=wqkv_ext[k * P : (k + 1) * P, :])
                nc.sync.dma_start(out=xo[k][:], in_=xo_ext[k * P : (k + 1) * P, :])
            for k in range(KD):
                nc.sync.dma_start(out=wh1_sb[k][:], in_=wh1_ext[k * P : (k + 1) * P, :])
            for k in range(2):
                nc.sync.dma_start(out=wh2_sb[k][:], in_=wh2_ext[k * P : (k + 1) * P, :])
                nc.sync.dma_start(out=bh1_sb[k][:], in_=bh1_ext[k * P : (k + 1) * P, :])
            nc.sync.dma_start(out=bh2_sb[:], in_=bh2_ext[:])
            nc.sync.dma_start(out=ch_sb[:], in_=ch_ext[:])
            nc.sync.dma_start(out=triu_sb[:], in_=triu_ext[:])
            with (
                tc.tile_pool(name="sqp", bufs=3) as sqp,
                tc.tile_pool(name="smallA", bufs=2) as smallA,
                tc.tile_pool(name="rbsp", bufs=3) as rbsp,
                tc.tile_pool(name="ssp", bufs=2, space="PSUM") as ssp,
                tc.tile_pool(name="qkvp", bufs=2, space="PSUM") as qkvp,
            ):
                for n in range(NCH):
                    ss_t = ssp.tile([P, 512], F32)
                    for k in range(KD):
                        sq = sqp.tile([P, 512], BF16)
                        if k % 2 == 0:
                            nc.scalar.square(sq[:], xh[k][n][:])
                        else:
                            nc.vector.tensor_mul(sq[:], xh[k][n][:], xh[k][n][:])
                        nc.tensor.matmul(
                            ss_t[:],
                            lhsT=ones_mat[:],
                            rhs=sq[:],
                            start=(k == 0),
                            stop=(k == KD - 1),
                        )
                    std_t = smallA.tile([P, 512], F32, tag="stdA")
                    nc.scalar.activation(
                        out=std_t[:], in_=ss_t[:], func=AF.Sqrt, bias=eps_sb[:], scale=1.0 / D
                    )
                    rstd_t = smallA.tile([P, 512], F32, tag="rstdA")
                    nc.vector.reciprocal_approx_fast(rstd_t[:], std_t[:])
                    rb_sb = rbsp.tile([P, 512], BF16)
                    nc.vector.tensor_copy(rb_sb[:], rstd_t[:])
                    for k in range(KD):
                        nc.vector.tensor_mul(xh[k][n][:], xh[k][n][:], rb_sb[:])

                # q, k (feature-major); k is written zero-padded per head so the
                # scores matmul can run with K=128 (block-diagonal stationary)
                for n in range(NCH):
                    for which in range(2):
                        ps = qkvp.tile([P, 512], F32, tag="qk_ps")
                        for k in range(KD):
                            nc.tensor.matmul(
                                ps[:],
                                lhsT=wqkv_sb[k][:, which * P : (which + 1) * P],
                                rhs=xh[k][n][:],
                                start=(k == 0),
                                stop=(k == KD - 1),
                            )
                        if which == 0:
                            nc.scalar.copy(qT[n][:], ps[:])
                        else:
                            nc.gpsimd.memset(kTz[0][n][64:128, :], 0.0)
                            nc.scalar.copy(kTz[0][n][0:64, :], ps[0:64, :])
                            nc.gpsimd.memset(kTz[1][n][0:64, :], 0.0)
                            nc.scalar.copy(kTz[1][n][64:128, :], ps[64:128, :])
                # v (token-major), zero-padded to 128 cols/head with a ones col
                for t in range(TT // P):
                    n, off = t // 4, (t % 4) * P
                    ps = qkvp.tile([P, P], F32, tag="v_ps")
                    for k in range(KD):
                        nc.tensor.matmul(
                            ps[:],
                            lhsT=xh[k][n][:, off : off + P],
                            rhs=wqkv_sb[k][:, 256:384],
                            start=(k == 0),
                            stop=(k == KD - 1),
                        )
                    v3 = vsb[t][:].rearrange("p (h x) -> p h x", h=2)
                    nc.scalar.copy(v3[:, :, 0:64], ps[:].rearrange("p (h x) -> p h x", h=2))
                    nc.gpsimd.memset(v3[:, :, 64:128], 1.0)
            xp_cm.__exit__(None, None, None)

            # ---- phase B: attention; A2A split per local head so the first
            # collective overlaps the second head's attention compute ----
            with (
                tc.tile_pool(name="expp", bufs=6) as expp,
                tc.tile_pool(name="smallB", bufs=2) as smallB,
                tc.tile_pool(name="sp", bufs=4, space="PSUM") as sp,
                tc.tile_pool(name="op", bufs=3, space="PSUM") as op,
            ):
                for hl in range(2):
                    hp = 64 * hl
                    for b in range(B):
                        for qc in range(NQC):
                            g_q = NQC * b + qc
                            o_ps = op.tile([P, 512], F32)
                            nkt = 4 * qc + 4
                            for kt in range(nkt):
                                tok0 = T * b + P * kt
                                g_k, off_k = tok0 // 512, tok0 % 512
                                s_ps = sp.tile([P, 512], F32)
                                nc.tensor.matmul(
                                    s_ps[:],
                                    lhsT=kTz[hl][g_k][:, off_k : off_k + P],
                                    rhs=qT[g_q][:],
                                    start=True,
                                    stop=True,
                                )
                                e_t = expp.tile([P, 512], BF16)
                                j = kt - 4 * qc
                                if j < 0:
                                    nc.scalar.activation(
                                        out=e_t[:], in_=s_ps[:], func=AF.Exp
                                    )
                                else:
                                    if j > 0:
                                        nc.vector.memset(e_t[:, 0 : P * j], 0.0)
                                    nc.scalar.activation(
                                        out=e_t[:, P * j : 512],
                                        in_=s_ps[:, P * j : 512],
                                        func=AF.Exp,
                                    )
                                    nc.vector.tensor_mul(
                                        e_t[:, P * j : P * (j + 1)],
                                        e_t[:, P * j : P * (j + 1)],
                                        triu_sb[:],
                                    )
                                nc.tensor.matmul(
                                    o_ps[:],
                                    lhsT=vsb[16 * b + kt][:, P * hl : P * (hl + 1)],
                                    rhs=e_t[:],
                                    start=(kt == 0),
                                    stop=(kt == nkt - 1),
                                )
                            den = smallB.tile([64, 512], F32, tag="den")
                            nc.vector.tensor_copy(den[:], o_ps[64:128, :])
                            r_t = smallB.tile([64, 512], F32, tag="r")
                            nc.vector.reciprocal_approx_fast(r_t[:], den[:])
                            ob = smallB.tile([64, 512], BF16, tag="ob")
                            nc.vector.tensor_copy(ob[:], o_ps[0:64, :])
                            nc.vector.tensor_mul(
                                oT[g_q][hp : hp + 64, :], ob[:], r_t[:]
                            )
                            if hl == 0:
                                nc.gpsimd.dma_start(
                                    out=a2aA_in[g_q], in_=oT[g_q][0:64, :]
                                )
                            else:
                                nc.gpsimd.dma_start(
                                    out=a2aB_in[g_q], in_=oT[g_q][64:128, :]
                                )
                    # resharding collective for this head-half
                    if hl == 0:
                        nc.gpsimd.collective_compute(
                            "AllToAll",
                            mybir.AluOpType.bypass,
                            replica_groups=[list(range(N_CORES))],
                            ins=[a2aA_in[:]],
                            outs=[a2aA_out[:]],
                        )
                        for k in range(KD):
                            nc.gpsimd.memset(oTfA[k][64:128, :], 0.0)
                            nc.gpsimd.dma_start(out=oTfA[k][0:64, :], in_=a2aA_out[k])
                    else:
                        nc.gpsimd.collective_compute(
                            "AllToAll",
                            mybir.AluOpType.bypass,
                            replica_groups=[list(range(N_CORES))],
                            ins=[a2aB_in[:]],
                            outs=[a2aB_out[:]],
                        )
                        for k in range(KD):
                            nc.gpsimd.memset(oTfB[k][0:64, :], 0.0)
                            nc.gpsimd.dma_start(out=oTfB[k][64:128, :], in_=a2aB_out[k])
            ab_cm.__exit__(None, None, None)

            # ---- phase C: out-proj + residual + rmsnorm2 ----
            with (
                tc.tile_pool(name="sq2p", bufs=3) as sq2p,
                tc.tile_pool(name="wosp", bufs=3) as wosp,
                tc.tile_pool(name="smallC", bufs=2) as smallC,
                tc.tile_pool(name="wop", bufs=2, space="PSUM") as wop,
                tc.tile_pool(name="ss2p", bufs=1, space="PSUM") as ss2p,
            ):
                for m in range(KD):
                    wot = wosp.tile([P, KD, P], BF16, tag="wos")
                    nc.sync.dma_start(out=wot[:], in_=wo_ext2[m])
                    w_ps = wop.tile([P, 512], F32)
                    for k in range(KD):
                        nc.tensor.matmul(
                            w_ps[:],
                            lhsT=wot[:, k, :],
                            rhs=oTfA[k][:],
                            start=(k == 0),
                            stop=(k == KD - 1),
                        )
                    nc.vector.tensor_add(xo[m][:], w_ps[:], xo[m][:])
                ss_t = ss2p.tile([P, 512], F32)
                for m in range(KD):
                    wot = wosp.tile([P, KD, P], BF16, tag="wosB")
                    nc.sync.dma_start(out=wot[:], in_=wo_ext2[m])
                    w_ps = wop.tile([P, 512], F32)
                    for k in range(KD):
                        nc.tensor.matmul(
                            w_ps[:],
                            lhsT=wot[:, k, :],
                            rhs=oTfB[k][:],
                            start=(k == 0),
                            stop=(k == KD - 1),
                        )
                    nc.vector.tensor_add(xo[m][:], w_ps[:], xo[m][:])
                    sq = sq2p.tile([P, 512], BF16)
                    if m % 2 == 0:
                        nc.scalar.square(sq[:], xo[m][:])
                    else:
                        nc.vector.tensor_mul(sq[:], xo[m][:], xo[m][:])
                    nc.tensor.matmul(
                        ss_t[:],
                        lhsT=ones_mat[:],
                        rhs=sq[:],
                        start=(m == 0),
                        stop=(m == KD - 1),
                    )
                std_t = smallC.tile([P, 512], F32, tag="stdC")
                nc.scalar.activation(
                    out=std_t[:], in_=ss_t[:], func=AF.Sqrt, bias=eps_sb[:], scale=1.0 / D
                )
                rstd_t = smallC.tile([P, 512], F32, tag="rstdC")
                nc.vector.reciprocal_approx_fast(rstd_t[:], std_t[:])
                rb_sb = smallC.tile([P, 512], BF16, tag="rb2sb")
                nc.vector.tensor_copy(rb_sb[:], rstd_t[:])
                for k in range(KD):
                    nc.vector.tensor_mul(h2[k][:], xo[k][:], rb_sb[:])

            # ---- phase D: FFN (full weights, streamed) ----
            with (
                tc.tile_pool(name="gffp", bufs=1) as gffp,
                tc.tile_pool(name="wffp", bufs=3) as wffp,
                tc.tile_pool(name="w3p", bufs=2) as w3p,
                tc.tile_pool(name="silp", bufs=3) as silp,
                tc.tile_pool(name="up", bufs=2, space="PSUM") as up,
                tc.tile_pool(name="w2psum", bufs=2, space="PSUM") as w2psum,
                tc.tile_pool(name="fp", bufs=2, space="PSUM") as fp,
            ):
                gff = [gffp.tile([P, TB], BF16, tag=f"gff{m}", name=f"gff{m}") for m in range(MFF)]
                for m in range(MFF):
                    w1t = wffp.tile([P, KD, P], BF16, tag="w1s")
                    nc.sync.dma_start(out=w1t[:], in_=w1_ext[m])
                    u_ps = up.tile([P, 512], F32)
                    for k in range(KD):
                        nc.tensor.matmul(
                            u_ps[:],
                            lhsT=w1t[:, k, :],
                            rhs=h2[k][:],
                            start=(k == 0),
                            stop=(k == KD - 1),
                        )
                    w2t = wffp.tile([P, KD, P], BF16, tag="w2s")
                    nc.sync.dma_start(out=w2t[:], in_=w2_ext[m])
                    w2_ps = w2psum.tile([P, 512], F32)
                    for k in range(KD):
                        nc.tensor.matmul(
                            w2_ps[:],
                            lhsT=w2t[:, k, :],
                            rhs=h2[k][:],
                            start=(k == 0),
                            stop=(k == KD - 1),
                        )
                    sil = silp.tile([P, 512], BF16)
                    nc.scalar.activation(out=sil[:], in_=u_ps[:], func=AF.Silu)
                    nc.vector.tensor_mul(gff[m][:], w2_ps[:], sil[:])
                for m3 in range(KD):
                    w3t = w3p.tile([P, MFF, P], BF16, tag="w3s")
                    nc.sync.dma_start(out=w3t[:], in_=w3_ext[m3])
                    f_ps = fp.tile([P, 512], F32)
                    for k3 in range(MFF):
                        nc.tensor.matmul(
                            f_ps[:],
                            lhsT=w3t[:, k3, :],
                            rhs=gff[k3][:],
                            start=(k3 == 0),
                            stop=(k3 == MFF - 1),
                        )
                    nc.vector.tensor_add(xo[m3][:], f_ps[:], xo[m3][:])
                    nc.vector.tensor_copy(h2[m3][:], xo[m3][:])  # x3 in bf16
                    nc.sync.dma_start(
                        out=xt_out[m3 * P : (m3 + 1) * P, :], in_=xo[m3][:]
                    )

            # ---- phase E: halting head ----
            with (
                tc.tile_pool(name="hps", bufs=2, space="PSUM") as hps,
                tc.tile_pool(name="hpp", bufs=1, space="PSUM") as hpp,
                tc.tile_pool(name="smallE", bufs=2) as smallE,
            ):
                hh = [smallE.tile([P, TB], BF16, tag=f"hh{k}", name=f"hh{k}") for k in range(2)]
                for mh in range(2):
                    h_ps = hps.tile([P, 512], F32)
                    for k in range(KD):
                        nc.tensor.matmul(
                            h_ps[:],
                            lhsT=wh1_sb[k][:, mh * P : (mh + 1) * P],
                            rhs=h2[k][:],
                            start=(k == 0),
                            stop=(k == KD - 1),
                        )
                    nc.scalar.activation(
                        out=hh[mh][:], in_=h_ps[:], func=AF.Gelu, bias=bh1_sb[mh][:]
                    )
                hp_ps = hpp.tile([1, 512], F32)
                for k2 in range(2):
                    nc.tensor.matmul(
                        hp_ps[:],
                        lhsT=wh2_sb[k2][:],
                        rhs=hh[k2][:],
                        start=(k2 == 0),
                        stop=(k2 == 1),
                    )
                nc.scalar.activation(
                    out=halt_sb[:], in_=hp_ps[:], func=AF.Sigmoid, bias=bh2_sb[:]
                )
                nc.vector.tensor_scalar(
                    out=still_sb[:],
                    in0=ch_sb[:],
                    scalar1=HALT_THRESHOLD,
                    scalar2=None,
                    op0=mybir.AluOpType.is_lt,
                )
                tmp = smallE.tile([1, TB], F32)
                nc.vector.tensor_mul(tmp[:], halt_sb[:], still_sb[:])
                nc.vector.tensor_add(ncum_sb[:], tmp[:], ch_sb[:])
                nc.sync.dma_start(out=halt_out[:], in_=halt_sb[:])
                nc.sync.dma_start(out=ncum_out[:], in_=ncum_sb[:])

    nc.compile()
    return nc


def _prep_in_maps(inputs):
    bf = ml_dtypes.bfloat16
    x = np.asarray(inputs["x"], np.float32)
    ch = np.asarray(inputs["cumulative_halt"], np.float32)
    g1 = np.asarray(inputs["g1"], np.float32)
    g2 = np.asarray(inputs["g2"], np.float32)
    Wqkv = np.asarray(inputs["Wqkv"], np.float32) * g1[:, None]
    Wo = np.asarray(inputs["Wo"], np.float32)
    W1 = np.asarray(inputs["W1"], np.float32) * g2[:, None]
    W2 = np.asarray(inputs["W2"], np.float32) * g2[:, None]
    W3 = np.asarray(inputs["W3"], np.float32)
    Wh1 = np.asarray(inputs["Wh1"], np.float32)
    bh1 = np.asarray(inputs["bh1"], np.float32)
    Wh2 = np.asarray(inputs["Wh2"], np.float32)
    bh2 = np.asarray(inputs["bh2"], np.float32)

    xt = np.ascontiguousarray(
        x.reshape(TT, D).T.reshape(KD, P, NCH, 512).transpose(2, 0, 1, 3)
    ).astype(bf)  # [NCH, KD, P, 512] chunk-major tiles

    W1p = np.zeros((D, DFFP), np.float32)
    W1p[:, :DFF] = W1
    W2p = np.zeros((D, DFFP), np.float32)
    W2p[:, :DFF] = W2
    W3p = np.zeros((DFFP, D), np.float32)
    W3p[:DFF, :] = W3
    w1t = np.ascontiguousarray(
        W1p.reshape(KD, P, MFF, P).transpose(2, 1, 0, 3)
    ).astype(bf)
    w2t = np.ascontiguousarray(
        W2p.reshape(KD, P, MFF, P).transpose(2, 1, 0, 3)
    ).astype(bf)
    w3t = np.ascontiguousarray(
        W3p.reshape(MFF, P, KD, P).transpose(2, 1, 0, 3)
    ).astype(bf)

    wo_bf = np.ascontiguousarray(
        Wo.reshape(KD, P, KD, P).transpose(2, 1, 0, 3)
    ).astype(bf)
    wh1_bf = Wh1.astype(bf)
    wh2_bf = Wh2.reshape(DH, 1).astype(bf)
    bh1_c = bh1.reshape(DH, 1).astype(np.float32)
    bh2_c = bh2.reshape(1, 1).astype(np.float32)
    triu = np.triu(np.ones((P, P), np.float32)).astype(bf)

    in_maps = []
    for c in range(N_CORES):
        b, blk = c // 4, c % 4
        # q/k/v columns for heads 2c, 2c+1; fold 1/8 score scale into Wq
        qcols = Wqkv[:, P * c : P * (c + 1)] * 0.125
        kcols = Wqkv[:, D + P * c : D + P * (c + 1)]
        vcols = Wqkv[:, 2 * D + P * c : 2 * D + P * (c + 1)]
        wqkv_c = np.concatenate([qcols, kcols, vcols], axis=1).astype(bf)
        xo_c = np.ascontiguousarray(
            x[b, TB * blk : TB * (blk + 1), :].T
        ).astype(np.float32)
        ch_c = np.ascontiguousarray(
            ch[b, TB * blk : TB * (blk + 1), 0].reshape(1, TB)
        ).astype(np.float32)
        in_maps.append(
            {
                "xt": xt,
                "xo": xo_c,
                "wqkv": wqkv_c,
                "wo": wo_bf,
                "w1": w1t,
                "w2": w2t,
                "w3": w3t,
                "wh1": wh1_bf,
                "bh1": bh1_c,
                "wh2": wh2_bf,
                "bh2": bh2_c,
                "ch": ch_c,
                "triu": triu,
            }
        )
    return in_maps


def kernel(**inputs):
    global LAST_EXEC_NS
    if "nc" not in _CACHE:
        _CACHE["nc"] = _build()
    nc = _CACHE["nc"]
    in_maps = _prep_in_maps(inputs)
    import os

    trace = bool(os.environ.get("KERNEL_TRACE"))
    res = run_bass_kernel_spmd(
        nc, in_maps, core_ids=list(range(N_CORES)), trace=trace
    )
    LAST_EXEC_NS = res.exec_time_ns
    _CACHE["last_results"] = res

    x_out = np.empty((B, T, D), np.float32)
    halt = np.empty((B, T, 1), np.float32)
    ncum = np.empty((B, T, 1), np.float32)
    for c in range(N_CORES):
        b, blk = c // 4, c % 4
        r = res.results[c]
        x_out[b, TB * blk : TB * (blk + 1), :] = r["xt_out"].T
        halt[b, TB * blk : TB * (blk + 1), 0] = r["halt_o"][0]
        ncum[b, TB * blk : TB * (blk + 1), 0] = r["ncum_o"][0]
    return (x_out, halt, ncum)


# revision 17
# speedup vs baseline: 1.1034x; 1.0440x over previous
"""Trainium2 Bass kernel for nn_AdaptiveTransformerBlock (B=2, T=2048, D=1024).

Strategy (8 NeuronCores, one chip):
  - x (both batches) is replicated to every core in bf16, transposed
    feature-major [D, B*T]. Every core computes rmsnorm1 + its 2 heads'
    q/k/v (head tensor-parallelism, 16 heads / 8 cores), and causal
    attention for those 2 heads over both batches.
  - One 8-rank AllToAll (1 MB) reshards attention output o from
    head-parallel to token-parallel (each core owns a 512-token block).
  - Out-proj, residual, rmsnorm2, SwiGLU FFN and the halting head then
    run fully locally on the core's 512 tokens with full (streamed)
    weights. Outputs are token-sharded; the host reassembles.

All matmuls run in bf16 with fp32 PSUM accumulation; the residual path
stays fp32. g1/g2 are folded into Wqkv/W1/W2 rows host-side; the 1/8
attention scale is folded into Wq.
"""

import sys

sys.path.insert(0, "/opt/trn_rl_repo")

import numpy as np
import ml_dtypes

import concourse.bass as bass
import concourse.mybir as mybir
import concourse.tile as tile
from concourse import bacc
from concourse.bass_utils import run_bass_kernel_spmd

BF16 = mybir.dt.bfloat16
F32 = mybir.dt.float32
AF = mybir.ActivationFunctionType

P = 128
D = 1024
B = 2
T = 2048
TT = B * T  # 4096
TB = 512  # tokens owned per core
KD = D // P  # 8
NCH = TT // 512  # 8 global 512-token chunks
NQC = T // 512  # 4 q-chunks per batch
DFF = 2730
DFFP = 2816  # padded to 22*128
MFF = DFFP // P  # 22
DH = 256
EPS = 1e-6
HALT_THRESHOLD = 0.99

N_CORES = 8

LAST_EXEC_NS = None
_CACHE = {}


def _build():
    nc = bacc.Bacc(
        "TRN2",
        target_bir_lowering=False,
        debug=False,
        enable_asserts=False,
        num_devices=N_CORES,
    )

    # ---- dram parameters ----
    xt_ext = nc.dram_tensor("xt", [NCH, KD, P, 512], BF16, kind="ExternalInput")
    xo_ext = nc.dram_tensor("xo", [D, TB], F32, kind="ExternalInput")
    wqkv_ext = nc.dram_tensor("wqkv", [D, 384], BF16, kind="ExternalInput")
    wo_ext2 = nc.dram_tensor("wo", [KD, P, KD, P], BF16, kind="ExternalInput")
    w1_ext = nc.dram_tensor("w1", [MFF, P, KD, P], BF16, kind="ExternalInput")
    w2_ext = nc.dram_tensor("w2", [MFF, P, KD, P], BF16, kind="ExternalInput")
    w3_ext = nc.dram_tensor("w3", [KD, P, MFF, P], BF16, kind="ExternalInput")
    wh1_ext = nc.dram_tensor("wh1", [D, DH], BF16, kind="ExternalInput")
    bh1_ext = nc.dram_tensor("bh1", [DH, 1], F32, kind="ExternalInput")
    wh2_ext = nc.dram_tensor("wh2", [DH, 1], BF16, kind="ExternalInput")
    bh2_ext = nc.dram_tensor("bh2", [1, 1], F32, kind="ExternalInput")
    ch_ext = nc.dram_tensor("ch", [1, TB], F32, kind="ExternalInput")
    triu_ext = nc.dram_tensor("triu", [P, P], BF16, kind="ExternalInput")

    xt_out = nc.dram_tensor("xt_out", [D, TB], F32, kind="ExternalOutput")
    halt_out = nc.dram_tensor("halt_o", [1, TB], F32, kind="ExternalOutput")
    ncum_out = nc.dram_tensor("ncum_o", [1, TB], F32, kind="ExternalOutput")

    warm_in = nc.dram_tensor("warm_in", [N_CORES, 1, 128], BF16)
    warm_out = nc.dram_tensor("warm_out", [N_CORES, 1, 128], BF16)
    a2aA_in = nc.dram_tensor("a2aA_in", [N_CORES, 64, TB], BF16)
    a2aA_out = nc.dram_tensor("a2aA_out", [N_CORES, 64, TB], BF16)
    a2aB_in = nc.dram_tensor("a2aB_in", [N_CORES, 64, TB], BF16)
    a2aB_out = nc.dram_tensor("a2aB_out", [N_CORES, 64, TB], BF16)

    with tile.TileContext(nc) as tc:
        with tc.tile_pool(name="pers", bufs=1) as pers:
            # persistent tiles (live across phases)
            xo = [pers.tile([P, TB], F32, tag=f"xo{k}", name=f"xo{k}") for k in range(KD)]
            wqkv_sb = [pers.tile([P, 384], BF16, tag=f"wqkv{k}", name=f"wqkv{k}") for k in range(KD)]
            wh1_sb = [pers.tile([P, DH], BF16, tag=f"wh1{k}", name=f"wh1{k}") for k in range(KD)]
            wh2_sb = [pers.tile([P, 1], BF16, tag=f"wh2{k}", name=f"wh2{k}") for k in range(2)]
            bh1_sb = [pers.tile([P, 1], F32, tag=f"bh1{k}", name=f"bh1{k}") for k in range(2)]
            bh2_sb = pers.tile([1, 1], F32, tag="bh2")
            ch_sb = pers.tile([1, TB], F32, tag="ch")
            triu_sb = pers.tile([P, P], BF16, tag="triu")
            ones_mat = pers.tile([P, P], BF16, tag="ones_mat")
            eps_sb = pers.tile([P, 1], F32, tag="eps")
            oTfA = [pers.tile([P, TB], BF16, tag=f"oTfA{k}", name=f"oTfA{k}") for k in range(KD)]
            oTfB = [pers.tile([P, TB], BF16, tag=f"oTfB{k}", name=f"oTfB{k}") for k in range(KD)]
            h2 = [pers.tile([P, TB], BF16, tag=f"h2{k}", name=f"h2{k}") for k in range(KD)]
            rstd2_t = pers.tile([P, TB], F32, tag="rstd2")
            halt_sb = pers.tile([1, TB], F32, tag="halt")
            still_sb = pers.tile([1, TB], F32, tag="still")
            ncum_sb = pers.tile([1, TB], F32, tag="ncum")

            # ---- attention-scope tensors (freed before the FFN phase) ----
            ab_cm = tc.tile_pool(name="ab", bufs=1)
            ab = ab_cm.__enter__()
            qT = [ab.tile([P, 512], BF16, tag=f"qT{g}", name=f"qT{g}") for g in range(NCH)]
            kTz = [
                [ab.tile([P, 512], BF16, tag=f"kTz{hl}_{g}", name=f"kTz{hl}_{g}") for g in range(NCH)]
                for hl in range(2)
            ]
            vsb = [ab.tile([P, 256], BF16, tag=f"v{t}", name=f"v{t}") for t in range(TT // P)]
            oT = [ab.tile([P, 512], BF16, tag=f"oT{g}", name=f"oT{g}") for g in range(NCH)]

            # ---- phase A: rmsnorm1 (in-place into xh) + qkv ----
            xp_cm = tc.tile_pool(name="xp", bufs=1)
            xp = xp_cm.__enter__()
            xh = [
                [xp.tile([P, 512], BF16, tag=f"xh{k}_{n}", name=f"xh{k}_{n}") for n in range(NCH)]
                for k in range(KD)
            ]
            # x tiles first: they gate the whole pipeline
            for n in range(NCH):
                for k in range(KD):
                    nc.sync.dma_start(out=xh[k][n][:], in_=xt_ext[n, k])
            nc.vector.memset(eps_sb[:], EPS)
            nc.vector.memset(ones_mat[:], 1.0)
            # tiny warm-up collective: absorbs first-collective setup cost and
            # aligns the cores while input DMAs stream
            warm_sb = pers.tile([1, 128], BF16, tag="warm")
            nc.vector.memset(warm_sb[:], 0.0)
            for g in range(N_CORES):
                nc.gpsimd.dma_start(out=warm_in[g], in_=warm_sb[:])
            nc.gpsimd.collective_compute(
                "AllToAll",
                mybir.AluOpType.bypass,
                replica_groups=[list(range(N_CORES))],
                ins=[warm_in[:]],
                outs=[warm_out[:]],
            )
            for k in range(KD):
                nc.sync.dma_start(out=wqkv_sb[k][:], in_=wqkv_ext[k * P : (k + 1) * P, :])
                nc.sync.dma_start(out=xo[k][:], in_=xo_ext[k * P : (k + 1) * P, :])
            for k in range(KD):
                nc.sync.dma_start(out=wh1_sb[k][:], in_=wh1_ext[k * P : (k + 1) * P, :])
            for k in range(2):
                nc.sync.dma_start(out=wh2_sb[k][:], in_=wh2_ext[k * P : (k + 1) * P, :])
                nc.sync.dma_start(out=bh1_sb[k][:], in_=bh1_ext[k * P : (k + 1) * P, :])
            nc.sync.dma_start(out=bh2_sb[:], in_=bh2_ext[:])
            nc.sync.dma_start(out=ch_sb[:], in_=ch_ext[:])
            nc.sync.dma_start(out=triu_sb[:], in_=triu_ext[:])
            with (
                tc.tile_pool(name="sqp", bufs=3) as sqp,
                tc.tile_pool(name="smallA", bufs=2) as smallA,
                tc.tile_pool(name="rbsp", bufs=8) as rbsp,
                tc.tile_pool(name="ssp", bufs=2, space="PSUM") as ssp,
                tc.tile_pool(name="rcp", bufs=2, space="PSUM") as rcp,
                tc.tile_pool(name="qkvp", bufs=2, space="PSUM") as qkvp,
            ):
                # qkv runs on RAW x; the rmsnorm scale rstd folds into the
                # PSUM->SBUF copies (per-column for q/k, per-partition for v),
                # so the norm-stats chain stays off the PE critical path.
                for n in range(NCH):
                    ss_t = ssp.tile([P, 512], F32)
                    for k in range(KD):
                        sq = sqp.tile([P, 512], BF16)
                        if k % 2 == 0:
                            nc.scalar.square(sq[:], xh[k][n][:])
                        else:
                            nc.vector.tensor_mul(sq[:], xh[k][n][:], xh[k][n][:])
                        nc.tensor.matmul(
                            ss_t[:],
                            lhsT=ones_mat[:],
                            rhs=sq[:],
                            start=(k == 0),
                            stop=(k == KD - 1),
                        )
                    std_t = smallA.tile([P, 512], F32, tag="stdA")
                    nc.scalar.activation(
                        out=std_t[:], in_=ss_t[:], func=AF.Sqrt, bias=eps_sb[:], scale=1.0 / D
                    )
                    rstd_t = smallA.tile([P, 512], F32, tag="rstdA")
                    nc.vector.reciprocal_approx_fast(rstd_t[:], std_t[:])
                    rstd_bf = smallA.tile([1, 512], BF16, tag="rstdAbf")
                    nc.vector.tensor_copy(rstd_bf[:], rstd_t[0:1, :])
                    # rstd as a [128,1] column per 128-token strip (for v scaling)
                    rcs = []
                    for c in range(4):
                        rc_ps = rcp.tile([P, 1], F32)
                        nc.tensor.matmul(
                            rc_ps[:],
                            lhsT=rstd_bf[0:1, c * P : (c + 1) * P],
                            rhs=ones_mat[0:1, 0:1],
                            start=True,
                            stop=True,
                        )
                        rc_sb = rbsp.tile([P, 1], F32, tag="rc")
                        nc.vector.tensor_copy(rc_sb[:], rc_ps[:])
                        rcs.append(rc_sb)
                    for which in range(2):
                        ps = qkvp.tile([P, 512], F32, tag="qk_ps")
                        for k in range(KD):
                            nc.tensor.matmul(
                                ps[:],
                                lhsT=wqkv_sb[k][:, which * P : (which + 1) * P],
                                rhs=xh[k][n][:],
                                start=(k == 0),
                                stop=(k == KD - 1),
                            )
                        if which == 0:
                            nc.vector.tensor_mul(qT[n][:], ps[:], rstd_t[:])
                        else:
                            nc.gpsimd.memset(kTz[0][n][64:128, :], 0.0)
                            nc.vector.tensor_mul(
                                kTz[0][n][0:64, :], ps[0:64, :], rstd_t[0:64, :]
                            )
                            nc.gpsimd.memset(kTz[1][n][0:64, :], 0.0)
                            nc.vector.tensor_mul(
                                kTz[1][n][64:128, :], ps[64:128, :], rstd_t[64:128, :]
                            )
                    for c in range(4):
                        t = 4 * n + c
                        ps = qkvp.tile([P, P], F32, tag="v_ps")
                        for k in range(KD):
                            nc.tensor.matmul(
                                ps[:],
                                lhsT=xh[k][n][:, c * P : (c + 1) * P],
                                rhs=wqkv_sb[k][:, 256:384],
                                start=(k == 0),
                                stop=(k == KD - 1),
                            )
                        v3 = vsb[t][:].rearrange("p (h x) -> p h x", h=2)
                        nc.scalar.mul(
                            v3[:, :, 0:64],
                            ps[:].rearrange("p (h x) -> p h x", h=2),
                            rcs[c][:],
                        )
                        nc.gpsimd.memset(v3[:, :, 64:128], 1.0)
            xp_cm.__exit__(None, None, None)

            # ---- phase B: attention; A2A split per local head so the first
            # collective overlaps the second head's attention compute ----
            with (
                tc.tile_pool(name="expp", bufs=6) as expp,
                tc.tile_pool(name="smallB", bufs=2) as smallB,
                tc.tile_pool(name="sp", bufs=4, space="PSUM") as sp,
                tc.tile_pool(name="op", bufs=3, space="PSUM") as op,
            ):
                for hl in range(2):
                    hp = 64 * hl
                    for b in range(B):
                        for qc in range(NQC):
                            g_q = NQC * b + qc
                            o_ps = op.tile([P, 512], F32)
                            nkt = 4 * qc + 4
                            for kt in range(nkt):
                                tok0 = T * b + P * kt
                                g_k, off_k = tok0 // 512, tok0 % 512
                                s_ps = sp.tile([P, 512], F32)
                                nc.tensor.matmul(
                                    s_ps[:],
                                    lhsT=kTz[hl][g_k][:, off_k : off_k + P],
                                    rhs=qT[g_q][:],
                                    start=True,
                                    stop=True,
                                )
                                e_t = expp.tile([P, 512], BF16)
                                j = kt - 4 * qc
                                if j < 0:
                                    nc.scalar.activation(
                                        out=e_t[:], in_=s_ps[:], func=AF.Exp
                                    )
                                else:
                                    if j > 0:
                                        nc.vector.memset(e_t[:, 0 : P * j], 0.0)
                                    nc.scalar.activation(
                                        out=e_t[:, P * j : 512],
                                        in_=s_ps[:, P * j : 512],
                                        func=AF.Exp,
                                    )
                                    nc.vector.tensor_mul(
                                        e_t[:, P * j : P * (j + 1)],
                                        e_t[:, P * j : P * (j + 1)],
                                        triu_sb[:],
                                    )
                                nc.tensor.matmul(
                                    o_ps[:],
                                    lhsT=vsb[16 * b + kt][:, P * hl : P * (hl + 1)],
                                    rhs=e_t[:],
                                    start=(kt == 0),
                                    stop=(kt == nkt - 1),
                                )
                            den = smallB.tile([64, 512], F32, tag="den")
                            nc.vector.tensor_copy(den[:], o_ps[64:128, :])
                            r_t = smallB.tile([64, 512], F32, tag="r")
                            nc.vector.reciprocal_approx_fast(r_t[:], den[:])
                            ob = smallB.tile([64, 512], BF16, tag="ob")
                            nc.vector.tensor_copy(ob[:], o_ps[0:64, :])
                            nc.vector.tensor_mul(
                                oT[g_q][hp : hp + 64, :], ob[:], r_t[:]
                            )
                            if hl == 0:
                                nc.gpsimd.dma_start(
                                    out=a2aA_in[g_q], in_=oT[g_q][0:64, :]
                                )
                            else:
                                nc.gpsimd.dma_start(
                                    out=a2aB_in[g_q], in_=oT[g_q][64:128, :]
                                )
                    # resharding collective for this head-half
                    if hl == 0:
                        nc.gpsimd.collective_compute(
                            "AllToAll",
                            mybir.AluOpType.bypass,
                            replica_groups=[list(range(N_CORES))],
                            ins=[a2aA_in[:]],
                            outs=[a2aA_out[:]],
                        )
                        for k in range(KD):
                            nc.gpsimd.memset(oTfA[k][64:128, :], 0.0)
                            nc.gpsimd.dma_start(out=oTfA[k][0:64, :], in_=a2aA_out[k])
                    else:
                        nc.gpsimd.collective_compute(
                            "AllToAll",
                            mybir.AluOpType.bypass,
                            replica_groups=[list(range(N_CORES))],
                            ins=[a2aB_in[:]],
                            outs=[a2aB_out[:]],
                        )
                        for k in range(KD):
                            nc.gpsimd.memset(oTfB[k][0:64, :], 0.0)
                            nc.gpsimd.dma_start(out=oTfB[k][64:128, :], in_=a2aB_out[k])
            ab_cm.__exit__(None, None, None)

            # ---- phase C: out-proj + residual + rmsnorm2 ----
            with (
                tc.tile_pool(name="sq2p", bufs=3) as sq2p,
                tc.tile_pool(name="wosp", bufs=3) as wosp,
                tc.tile_pool(name="smallC", bufs=2) as smallC,
                tc.tile_pool(name="wop", bufs=2, space="PSUM") as wop,
                tc.tile_pool(name="ss2p", bufs=1, space="PSUM") as ss2p,
            ):
                for m in range(KD):
                    wot = wosp.tile([P, KD, P], BF16, tag="wos")
                    nc.sync.dma_start(out=wot[:], in_=wo_ext2[m])
                    w_ps = wop.tile([P, 512], F32)
                    for k in range(KD):
                        nc.tensor.matmul(
                            w_ps[:],
                            lhsT=wot[:, k, :],
                            rhs=oTfA[k][:],
                            start=(k == 0),
                            stop=(k == KD - 1),
                        )
                    nc.vector.tensor_add(xo[m][:], w_ps[:], xo[m][:])
                ss_t = ss2p.tile([P, 512], F32)
                for m in range(KD):
                    wot = wosp.tile([P, KD, P], BF16, tag="wosB")
                    nc.sync.dma_start(out=wot[:], in_=wo_ext2[m])
                    w_ps = wop.tile([P, 512], F32)
                    for k in range(KD):
                        nc.tensor.matmul(
                            w_ps[:],
                            lhsT=wot[:, k, :],
                            rhs=oTfB[k][:],
                            start=(k == 0),
                            stop=(k == KD - 1),
                        )
                    nc.vector.tensor_add(xo[m][:], w_ps[:], xo[m][:])
                    sq = sq2p.tile([P, 512], BF16)
                    if m % 2 == 0:
                        nc.scalar.square(sq[:], xo[m][:])
                    else:
                        nc.vector.tensor_mul(sq[:], xo[m][:], xo[m][:])
                    nc.tensor.matmul(
                        ss_t[:],
                        lhsT=ones_mat[:],
                        rhs=sq[:],
                        start=(m == 0),
                        stop=(m == KD - 1),
                    )
                std_t = smallC.tile([P, 512], F32, tag="stdC")
                nc.scalar.activation(
                    out=std_t[:], in_=ss_t[:], func=AF.Sqrt, bias=eps_sb[:], scale=1.0 / D
                )
                nc.vector.reciprocal_approx_fast(rstd2_t[:], std_t[:])
                # h2 holds bf16 x2 (unnormalized); rstd2 is applied inside the
                # SwiGLU elementwise ops, off the PE critical path
                for k in range(KD):
                    nc.vector.tensor_copy(h2[k][:], xo[k][:])

            # ---- phase D: FFN (full weights, streamed) ----
            with (
                tc.tile_pool(name="gffp", bufs=1) as gffp,
                tc.tile_pool(name="wffp", bufs=3) as wffp,
                tc.tile_pool(name="w3p", bufs=2) as w3p,
                tc.tile_pool(name="silp", bufs=3) as silp,
                tc.tile_pool(name="up", bufs=2, space="PSUM") as up,
                tc.tile_pool(name="w2psum", bufs=2, space="PSUM") as w2psum,
                tc.tile_pool(name="fp", bufs=2, space="PSUM") as fp,
            ):
                gff = [gffp.tile([P, TB], BF16, tag=f"gff{m}", name=f"gff{m}") for m in range(MFF)]
                for m in range(MFF):
                    w1t = wffp.tile([P, KD, P], BF16, tag="w1s")
                    nc.sync.dma_start(out=w1t[:], in_=w1_ext[m])
                    u_ps = up.tile([P, 512], F32)
                    for k in range(KD):
                        nc.tensor.matmul(
                            u_ps[:],
                            lhsT=w1t[:, k, :],
                            rhs=h2[k][:],
                            start=(k == 0),
                            stop=(k == KD - 1),
                        )
                    w2t = wffp.tile([P, KD, P], BF16, tag="w2s")
                    nc.sync.dma_start(out=w2t[:], in_=w2_ext[m])
                    w2_ps = w2psum.tile([P, 512], F32)
                    for k in range(KD):
                        nc.tensor.matmul(
                            w2_ps[:],
                            lhsT=w2t[:, k, :],
                            rhs=h2[k][:],
                            start=(k == 0),
                            stop=(k == KD - 1),
                        )
                    usc = silp.tile([P, 512], F32, tag="usc")
                    nc.vector.tensor_mul(usc[:], u_ps[:], rstd2_t[:])
                    sil = silp.tile([P, 512], BF16, tag="sil")
                    nc.scalar.activation(out=sil[:], in_=usc[:], func=AF.Silu)
                    gt = silp.tile([P, 512], BF16, tag="gt")
                    nc.vector.tensor_mul(gt[:], w2_ps[:], sil[:])
                    nc.vector.tensor_mul(gff[m][:], gt[:], rstd2_t[:])
                for m3 in range(KD):
                    w3t = w3p.tile([P, MFF, P], BF16, tag="w3s")
                    nc.sync.dma_start(out=w3t[:], in_=w3_ext[m3])
                    f_ps = fp.tile([P, 512], F32)
                    for k3 in range(MFF):
                        nc.tensor.matmul(
                            f_ps[:],
                            lhsT=w3t[:, k3, :],
                            rhs=gff[k3][:],
                            start=(k3 == 0),
                            stop=(k3 == MFF - 1),
                        )
                    nc.vector.tensor_add(xo[m3][:], f_ps[:], xo[m3][:])
                    nc.vector.tensor_copy(h2[m3][:], xo[m3][:])  # x3 in bf16
                    nc.sync.dma_start(
                        out=xt_out[m3 * P : (m3 + 1) * P, :], in_=xo[m3][:]
                    )

            # ---- phase E: halting head ----
            with (
                tc.tile_pool(name="hps", bufs=2, space="PSUM") as hps,
                tc.tile_pool(name="hpp", bufs=1, space="PSUM") as hpp,
                tc.tile_pool(name="smallE", bufs=2) as smallE,
            ):
                hh = [smallE.tile([P, TB], BF16, tag=f"hh{k}", name=f"hh{k}") for k in range(2)]
                for mh in range(2):
                    h_ps = hps.tile([P, 512], F32)
                    for k in range(KD):
                        nc.tensor.matmul(
                            h_ps[:],
                            lhsT=wh1_sb[k][:, mh * P : (mh + 1) * P],
                            rhs=h2[k][:],
                            start=(k == 0),
                            stop=(k == KD - 1),
                        )
                    nc.scalar.activation(
                        out=hh[mh][:], in_=h_ps[:], func=AF.Gelu, bias=bh1_sb[mh][:]
                    )
                hp_ps = hpp.tile([1, 512], F32)
                for k2 in range(2):
                    nc.tensor.matmul(
                        hp_ps[:],
                        lhsT=wh2_sb[k2][:],
                        rhs=hh[k2][:],
                        start=(k2 == 0),
                        stop=(k2 == 1),
                    )
                nc.scalar.activation(
                    out=halt_sb[:], in_=hp_ps[:], func=AF.Sigmoid, bias=bh2_sb[:]
                )
                nc.vector.tensor_scalar(
                    out=still_sb[:],
                    in0=ch_sb[:],
                    scalar1=HALT_THRESHOLD,
                    scalar2=None,
                    op0=mybir.AluOpType.is_lt,
                )
                tmp = smallE.tile([1, TB], F32)
                nc.vector.tensor_mul(tmp[:], halt_sb[:], still_sb[:])
                nc.vector.tensor_add(ncum_sb[:], tmp[:], ch_sb[:])
                nc.sync.dma_start(out=halt_out[:], in_=halt_sb[:])
                nc.sync.dma_start(out=ncum_out[:], in_=ncum_sb[:])

    nc.compile()
    return nc


def _prep_in_maps(inputs):
    bf = ml_dtypes.bfloat16
    x = np.asarray(inputs["x"], np.float32)
    ch = np.asarray(inputs["cumulative_halt"], np.float32)
    g1 = np.asarray(inputs["g1"], np.float32)
    g2 = np.asarray(inputs["g2"], np.float32)
    Wqkv = np.asarray(inputs["Wqkv"], np.float32) * g1[:, None]
    Wo = np.asarray(inputs["Wo"], np.float32)
    W1 = np.asarray(inputs["W1"], np.float32) * g2[:, None]
    W2 = np.asarray(inputs["W2"], np.float32) * g2[:, None]
    W3 = np.asarray(inputs["W3"], np.float32)
    Wh1 = np.asarray(inputs["Wh1"], np.float32)
    bh1 = np.asarray(inputs["bh1"], np.float32)
    Wh2 = np.asarray(inputs["Wh2"], np.float32)
    bh2 = np.asarray(inputs["bh2"], np.float32)

    xt = np.ascontiguousarray(
        x.reshape(TT, D).T.reshape(KD, P, NCH, 512).transpose(2, 0, 1, 3)
    ).astype(bf)  # [NCH, KD, P, 512] chunk-major tiles

    W1p = np.zeros((D, DFFP), np.float32)
    W1p[:, :DFF] = W1
    W2p = np.zeros((D, DFFP), np.float32)
    W2p[:, :DFF] = W2
    W3p = np.zeros((DFFP, D), np.float32)
    W3p[:DFF, :] = W3
    w1t = np.ascontiguousarray(
        W1p.reshape(KD, P, MFF, P).transpose(2, 1, 0, 3)
    ).astype(bf)
    w2t = np.ascontiguousarray(
        W2p.reshape(KD, P, MFF, P).transpose(2, 1, 0, 3)
    ).astype(bf)
    w3t = np.ascontiguousarray(
        W3p.reshape(MFF, P, KD, P).transpose(2, 1, 0, 3)
    ).astype(bf)

    wo_bf = np.ascontiguousarray(
        Wo.reshape(KD, P, KD, P).transpose(2, 1, 0, 3)
    ).astype(bf)
    wh1_bf = Wh1.astype(bf)
    wh2_bf = Wh2.reshape(DH, 1).astype(bf)
    bh1_c = bh1.reshape(DH, 1).astype(np.float32)
    bh2_c = bh2.reshape(1, 1).astype(np.float32)
    triu = np.triu(np.ones((P, P), np.float32)).astype(bf)

    in_maps = []
    for c in range(N_CORES):
        b, blk = c // 4, c % 4
        # q/k/v columns for heads 2c, 2c+1; fold 1/8 score scale into Wq
        qcols = Wqkv[:, P * c : P * (c + 1)] * 0.125
        kcols = Wqkv[:, D + P * c : D + P * (c + 1)]
        vcols = Wqkv[:, 2 * D + P * c : 2 * D + P * (c + 1)]
        wqkv_c = np.concatenate([qcols, kcols, vcols], axis=1).astype(bf)
        xo_c = np.ascontiguousarray(
            x[b, TB * blk : TB * (blk + 1), :].T
        ).astype(np.float32)
        ch_c = np.ascontiguousarray(
            ch[b, TB * blk : TB * (blk + 1), 0].reshape(1, TB)
        ).astype(np.float32)
        in_maps.append(
            {
                "xt": xt,
                "xo": xo_c,
                "wqkv": wqkv_c,
                "wo": wo_bf,
                "w1": w1t,
                "w2": w2t,
                "w3": w3t,
                "wh1": wh1_bf,
                "bh1": bh1_c,
                "wh2": wh2_bf,
                "bh2": bh2_c,
                "ch": ch_c,
                "triu": triu,
            }
        )
    return in_maps


def kernel(**inputs):
    global LAST_EXEC_NS
    if "nc" not in _CACHE:
        _CACHE["nc"] = _build()
    nc = _CACHE["nc"]
    in_maps = _prep_in_maps(inputs)
    import os

    trace = bool(os.environ.get("KERNEL_TRACE"))
    res = run_bass_kernel_spmd(
        nc, in_maps, core_ids=list(range(N_CORES)), trace=trace
    )
    LAST_EXEC_NS = res.exec_time_ns
    _CACHE["last_results"] = res

    x_out = np.empty((B, T, D), np.float32)
    halt = np.empty((B, T, 1), np.float32)
    ncum = np.empty((B, T, 1), np.float32)
    for c in range(N_CORES):
        b, blk = c // 4, c % 4
        r = res.results[c]
        x_out[b, TB * blk : TB * (blk + 1), :] = r["xt_out"].T
        halt[b, TB * blk : TB * (blk + 1), 0] = r["halt_o"][0]
        ncum[b, TB * blk : TB * (blk + 1), 0] = r["ncum_o"][0]
    return (x_out, halt, ncum)
